# revision 40
# baseline (speedup 1.0000x reference)
"""Trainium2 Bass kernel for the MoE-routing random-feature ridge problem.

Strategy (8 NeuronCores, atom-sharded phase 1 + covering-design phase 2):
  - Molecules are assigned to cores (128 each) by a greedy balance of
    per-element atom counts, so each core gets ~512 atoms of each element
    and elem groups pad to T ~ 19 tiles of 128 atoms.
  - Phase 1 per core, in 4 feature-quarter passes (1024 cols each):
      PT  = reductors[e]^T @ gto^T    bf16 -> fp8     [256, 128] per tile
      PW  = PT^T @ W[e] quarter       fp8 DoubleRow   [128, 1024] psum
      fw  = wrap(PW + c) into [-pi,pi]  custom DVE    bf16
      F   = sin(fw)                   ScalarE         fp8
      Z  += ST^T @ F per tile-pair    fp8 DoubleRow   psum accumulate
    where c = wrap(b + pi/2), so sin(x + c) = cos(x + b).
    Z spills straight to fp8 on GPSIMD (keeps ScalarE for sins); the
    per-quarter zty slice comes from the fp8 Z via tiny PE matmuls with
    fp8 y, and rides as 8 extra bitcast rows in quarter 3's AllGather.
  - Per-quarter AllGathers are triggered from gpsimd as soon as each z8
    spill lands; ag_all concat DMAs ride the ScalarE DMA queue so they
    don't head-of-line-block the spills.
  - Phase 2: each core d reads the feature slices {d, d+1, d+2, d+4}
    (mod 8) of the full Z (fp8) via partition-id dynamic APs and runs the
    same program: 5 [512,512] blocks of Z^T Z with fp8 DoubleRow over 4
    mol-tile pairs; the 8 slice quadruples cover all 36 upper-triangle
    blocks; ZTZ psum DMAs straight to DRAM; host mirrors.
  - Host applies scale^2 = 2/NFEAT, adds lambda*I, assembles + mirrors.
"""

import sys

if "/opt/trn_rl_repo" not in sys.path:
    sys.path.insert(0, "/opt/trn_rl_repo")

import numpy as np

import concourse.bacc as bacc
import concourse.mybir as mybir
import concourse.tile as tile
from concourse import bass_utils
from concourse.ap import AP

NCORES = 8
NATOMS = 16384
NMOL = 1024
REP = 512
PROJ = 256
NFEAT = 4096
NELEM = 4
LLAMBDA = 1e-6
MPC = NMOL // NCORES      # mols per core (128)
NQ = 4                    # feature quarters
QF = NFEAT // NQ          # 1024
DELTAS = (0, 1, 4, 2)
POS_BLOCKS = [(0, 0), (0, 1), (1, 2), (2, 3), (0, 2)]
NBLK = len(POS_BLOCKS)
ZR = 8                    # zty payload rows (bf16 [1,4096] as fp8 [8,1024])
CR = MPC + ZR             # quarter-3 a2a rows

F32 = mybir.dt.float32
BF16 = mybir.dt.bfloat16
FP8 = mybir.dt.float8e4
NP_FP8 = mybir.dt.np(FP8)
NP_BF16 = mybir.dt.np(BF16)

# --- fused (in0 + in1) + range-wrap custom DVE op ---------------------------
from concourse import dve_ops as _dve_ops
from concourse.dve_spec import Spec as _Spec, Src0 as _Src0, Src1 as _Src1
from concourse.dve_spec import C1 as _C1, C2 as _C2, _has_src1, lower as _dve_lower
from concourse.dve_uop import DveOpSpec as _DveOpSpec

_A2RW_NAME = "ADD2_RANGE_WRAP_ANT"
if _A2RW_NAME not in _dve_ops._SUB_OPCODE_FOR_NAME:
    _y = _Src0 + _Src1
    _a2_spec = _Spec(
        body=_y + _C2 * ((_y < -_C1) - (_y > _C1)),
        reference=lambda in0, in1, s0, s1, imm2: (in0 + in1)
        + imm2
        * (
            ((in0 + in1) < -s1).astype(np.float32)
            - ((in0 + in1) > s1).astype(np.float32)
        ),
    )
    _shas = {}
    for _ver in ("v3", "v4"):
        _tmp = _DveOpSpec(name=_A2RW_NAME, opcode=1,
                          uops=_dve_lower(_a2_spec, ver=_ver),
                          rd1_en=_has_src1(_a2_spec))
        _shas[_ver] = _tmp.sha(_ver)
    ADD2_RANGE_WRAP = _dve_ops.DveOp(_A2RW_NAME, _a2_spec, subdim=False, uops_sha=_shas)
    _dve_ops.OPS.append(ADD2_RANGE_WRAP)
    _dve_ops.CUSTOM_DVE_SPECS[_A2RW_NAME] = _a2_spec
    _dve_ops._SUB_OPCODE_FOR_NAME[_A2RW_NAME] = (
        max(_dve_ops._SUB_OPCODE_FOR_NAME.values()) + 1
    )
else:
    ADD2_RANGE_WRAP = next(o for o in _dve_ops.OPS if o.name == _A2RW_NAME)

_cache = {}


def _plan(charges, molIDs):
    charges = np.asarray(charges)
    molIDs = np.asarray(molIDs)
    assert np.all(np.diff(molIDs) >= 0)

    cnt = np.zeros((NMOL, NELEM), np.int64)
    np.add.at(cnt, (molIDs, charges), 1)

    # greedy balanced mol->core assignment (capacity 128 mols per core)
    order = np.argsort(-cnt.sum(1), kind="stable")
    load = np.zeros((NCORES, NELEM), np.float64)
    nmol = np.zeros(NCORES, np.int64)
    core_of = np.zeros(NMOL, np.int64)
    for m in order:
        best, bestJ = -1, None
        for c in range(NCORES):
            if nmol[c] >= MPC:
                continue
            J = float(((load[c] + cnt[m]) ** 2).sum())
            if bestJ is None or J < bestJ:
                best, bestJ = c, J
        core_of[m] = best
        load[best] += cnt[m]
        nmol[best] += 1
    assert np.all(nmol == MPC)

    # hill-climb refinement: swap mols between cores to reduce
    # T = sum_e max_c ceil(cnt[c,e]/128), tiebreak sum_e max_c cnt[c,e]
    icnt = np.zeros((NCORES, NELEM), np.int64)
    for c in range(NCORES):
        icnt[c] = cnt[core_of == c].sum(axis=0)

    def loss(ic):
        mx = ic.max(axis=0)
        return (int(np.ceil(mx / 128.0).sum()) * 1000000 + int(mx.sum()))

    rng = np.random.default_rng(12345)
    cur = loss(icnt)
    mols_by_core = [list(np.nonzero(core_of == c)[0]) for c in range(NCORES)]
    for _ in range(20000):
        c1, c2 = rng.integers(0, NCORES, 2)
        if c1 == c2:
            continue
        m1 = mols_by_core[c1][int(rng.integers(0, MPC))]
        m2 = mols_by_core[c2][int(rng.integers(0, MPC))]
        d1, d2 = cnt[m1], cnt[m2]
        icnt[c1] += d2 - d1
        icnt[c2] += d1 - d2
        new = loss(icnt)
        if new <= cur:
            cur = new
            core_of[m1], core_of[m2] = c2, c1
            mols_by_core[c1].remove(m1); mols_by_core[c1].append(m2)
            mols_by_core[c2].remove(m2); mols_by_core[c2].append(m1)
        else:
            icnt[c1] -= d2 - d1
            icnt[c2] -= d1 - d2
    core_mols = [np.nonzero(core_of == c)[0] for c in range(NCORES)]

    # per-core per-element atom lists and global tile counts
    icnt = np.zeros((NCORES, NELEM), np.int64)
    for c in range(NCORES):
        for e in range(NELEM):
            icnt[c, e] = int(cnt[core_mols[c], e].sum())
    T_e = [int(np.ceil(icnt[:, e].max() / 128)) for e in range(NELEM)]
    T = sum(T_e)
    FP = T // 2          # full DoubleRow pairs
    P = (T + 1) // 2     # st8 pair-slot count
    tile_elem = []
    for e in range(NELEM):
        tile_elem += [e] * T_e[e]

    # proj chunks: runs of same-element tiles, up to 4 tiles each
    chunks = []  # (t0, L, e)
    t = 0
    for e in range(NELEM):
        left = T_e[e]
        while left > 0:
            L = min(4, left)
            chunks.append((t, L, e))
            t += L
            left -= L

    # per-core slot table [T*128] -> atom index or -1; local mol index
    slot_atom = np.full((NCORES, T * 128), -1, np.int64)
    mol_loc = np.full(NMOL, -1, np.int64)
    for c in range(NCORES):
        for i, m in enumerate(core_mols[c]):
            mol_loc[m] = i
        t0 = 0
        core_mask = core_of[molIDs] == c
        for e in range(NELEM):
            idx = np.nonzero(core_mask & (charges == e))[0]
            slot_atom[c, t0 * 128: t0 * 128 + len(idx)] = idx
            t0 += T_e[e]

    # ST (fp8) per pair: [P, 128, 2*128]
    st8 = np.zeros((NCORES, P, 128, 256), dtype=NP_FP8)
    for c in range(NCORES):
        sl = slot_atom[c]
        real = np.nonzero(sl >= 0)[0]
        ml = mol_loc[molIDs[sl[real]]]
        tt = real // 128
        ii = real % 128
        st8[c, tt // 2, ii, (tt % 2) * 128 + ml] = 1.0

    # covering design + host assembly map
    S = [[(d + dl) % NCORES for dl in DELTAS] for d in range(NCORES)]
    cover = set()
    for d in range(NCORES):
        for (a, b) in POS_BLOCKS:
            i, j = S[d][a], S[d][b]
            cover.add((min(i, j), max(i, j)))
    assert len(cover) == 36, f"coverage {len(cover)}"

    return dict(core_mols=core_mols, T_e=T_e, T=T, P=P, FP=FP,
                tile_elem=tile_elem, chunks=chunks, slot_atom=slot_atom,
                S=S, st8=st8)


def _build(plan):
    T, P = plan["T"], plan["P"]
    FP = plan["FP"]
    tile_elem = plan["tile_elem"]
    chunks = plan["chunks"]

    nc = bacc.Bacc(num_devices=NCORES)
    gto_d = nc.dram_tensor("gto_swz", [128, T * 512], BF16, kind="ExternalInput")
    st_d = nc.dram_tensor("st_swz", [128, P * 256], FP8, kind="ExternalInput")
    red_d = nc.dram_tensor("red_swz", [128, NELEM * 4 * 2 * 128], BF16,
                           kind="ExternalInput")
    w_d = nc.dram_tensor("w_swz", [128, NELEM * 2 * NFEAT], FP8,
                         kind="ExternalInput")
    c_d = nc.dram_tensor("c_swz", [NQ, 1, NELEM * QF], BF16, kind="ExternalInput")
    y_d = nc.dram_tensor("y_swz", [128, 1], BF16, kind="ExternalInput")
    ztz_d = nc.dram_tensor("ztz", [NBLK * 4 * 128, 512], F32, kind="ExternalOutput")
    zty_d = nc.dram_tensor("zty", [1, NFEAT], BF16, kind="ExternalOutput")

    PI = float(np.pi)
    e0 = tile_elem[0]

    with tile.TileContext(nc) as tc:
        with (
            tc.tile_pool(name="const", bufs=1) as constp,
            tc.tile_pool(name="zacc", bufs=1) as zaccp,
            tc.tile_pool(name="dram", bufs=1, space="DRAM") as dramp,
        ):
            # priority order: everything the first PW tile needs comes first.
            # c broadcasts are 0-stride replicating DMAs (gpsimd is still
            # busy loading its library this early)
            def bcast_c(dst_ap, q, col0, cols, eng):
                src = AP(tensor=c_d, offset=q * (NELEM * QF) + col0,
                         ap=[[0, 128], [1, cols]])
                eng.dma_start(out=dst_ap, in_=src)

            # sync ring: gto (gates PT) first, then red, then the rest
            gto_sb = constp.tile([128, T * 512], BF16, tag="gto")
            nc.sync.dma_start(out=gto_sb[:, 0:4 * 512], in_=gto_d[:, 0:4 * 512])
            red_sb = constp.tile([128, NELEM * 4 * 2 * 128], BF16, tag="red")
            nc.sync.dma_start(out=red_sb[:], in_=red_d[:])
            nc.sync.dma_start(out=gto_sb[:, 4 * 512:], in_=gto_d[:, 4 * 512:])
            # scalar ring: c broadcast (first wrap) + quarter-0 weights
            c_bcq0 = constp.tile([128, NELEM * QF], BF16, tag="cbq0", name="cbq0")
            c_bcqs = [c_bcq0]
            bcast_c(c_bcq0[:, e0 * QF:(e0 + 1) * QF], 0, e0 * QF, QF, nc.scalar)
            w8_sb = constp.tile([128, NELEM * 2 * NFEAT], FP8, tag="w8")
            for pl in range(2):
                base = (e0 * 2 + pl) * NFEAT
                nc.scalar.dma_start(out=w8_sb[:, base:base + QF],
                                    in_=w_d[:, base:base + QF])
            st8_sb = constp.tile([128, P * 256], FP8, tag="st8")
            nc.scalar.dma_start(out=st8_sb[:, 0:512], in_=st_d[:, 0:512])
            # warmup collective: absorbs first-collective setup + core skew
            warm_in = dramp.tile([1, 64], FP8, tag="warmin")
            warm_out = dramp.tile([NCORES, 64], FP8, addr_space="Shared",
                                  tag="warmout", name="warm_b")
            nc.gpsimd.collective_compute(
                "AllGather", mybir.AluOpType.bypass,
                replica_groups=[list(range(NCORES))],
                ins=[warm_in[:].opt()],
                outs=[warm_out[:].opt()],
            )
            # rest of quarter 0 weights + remaining c broadcasts
            for e in range(NELEM):
                if e == e0:
                    continue
                for pl in range(2):
                    base = (e * 2 + pl) * NFEAT
                    nc.scalar.dma_start(out=w8_sb[:, base:base + QF],
                                        in_=w_d[:, base:base + QF])
            for e in range(NELEM):
                if e == e0:
                    continue
                bcast_c(c_bcq0[:, e * QF:(e + 1) * QF], 0, e * QF, QF,
                        nc.scalar)
            # later-quarter broadcasts on gpsimd (free once its library loads)
            for q in range(1, NQ):
                c_sbq = constp.tile([1, NELEM * QF], BF16, tag=f"csq{q}",
                                    name=f"csq{q}")
                nc.scalar.dma_start(out=c_sbq[:], in_=c_d[q, :, :])
                c_bcq = constp.tile([128, NELEM * QF], BF16, tag=f"cbq{q}",
                                    name=f"cbq{q}")
                nc.gpsimd.partition_broadcast(c_bcq[:], c_sbq[:])
                c_bcqs.append(c_bcq)
            y_sb = constp.tile([128, 1], BF16, tag="ybf")
            nc.scalar.dma_start(out=y_sb[:], in_=y_d[:])
            nc.sync.dma_start(out=st8_sb[:, 512:], in_=st_d[:, 512:])
            for e in range(NELEM):
                for pl in range(2):
                    base = (e * 2 + pl) * NFEAT
                    eng = nc.sync if e < 2 else nc.scalar
                    eng.dma_start(out=w8_sb[:, base + QF:base + NFEAT],
                                  in_=w_d[:, base + QF:base + NFEAT])

            pt8_sb = zaccp.tile([128, T * 256], FP8, tag="pt8")
            zty_bf = zaccp.tile([1, NFEAT], BF16, tag="ztybf")

            ag_ins, ag_outs = [], []
            for q in range(NQ):
                ag_ins.append(dramp.tile([MPC, QF], FP8, tag=f"agin{q}",
                                         name=f"agin{q}"))
                ag_outs.append(dramp.tile([NCORES * MPC, QF], FP8,
                                          addr_space="Shared", tag=f"agout{q}",
                                          name=f"ag_b{q}"))
            # mod-8 dynamic addressing in phase 2: no wraparound dups needed
            ag_all = dramp.tile([NMOL, 4 * 1024], FP8, tag="agall")
            ary_in = dramp.tile([1, NFEAT], BF16, tag="aryin")
            ary_out = dramp.tile([1, NFEAT], BF16, addr_space="Shared",
                                 tag="aryout", name="ary_b")

            DR = mybir.MatmulPerfMode.DoubleRow

            with (
                tc.tile_pool(name="pw", bufs=2, space="PSUM") as pwp,
                tc.tile_pool(name="fw", bufs=6) as fwp,
                tc.tile_pool(name="f8", bufs=3) as f8p,
                tc.tile_pool(name="z8", bufs=2) as z8p,
                tc.tile_pool(name="zbf", bufs=4) as zbfp,
            ):
                def emit_zty(q, z_bf, zpool):
                    # borrow a freed z-slot; tiny PE mms from the bf16 Z
                    zty_ps = zpool.tile([128, QF], F32, tag="z",
                                        name=f"ztyps{q}")
                    for h in range(2):
                        nc.tensor.matmul(
                            zty_ps[0:1, h * 512:(h + 1) * 512],
                            y_sb[:], z_bf[:, h * 512:(h + 1) * 512],
                            start=True, stop=True,
                        )
                    nc.scalar.copy(zty_bf[:, q * QF:(q + 1) * QF],
                                   zty_ps[0:1, 0:QF])

                def quarter(q, zpool, ptp=None):
                    c_bcq = c_bcqs[q]
                    z_ps = zpool.tile([128, QF], F32, tag="z", name=f"z_q{q}")
                    if q == 0:
                        for ci, (t0, L, e) in enumerate(chunks):
                            for mp in range(2):
                                pt_ps = ptp.tile([128, 512], F32, tag="pt")
                                for kt in range(4):
                                    nc.tensor.matmul(
                                        pt_ps[:, 0:L * 128],
                                        red_sb[:, ((e * 4 + kt) * 2 + mp) * 128:
                                                  ((e * 4 + kt) * 2 + mp) * 128 + 128],
                                        gto_sb[:, t0 * 512:(t0 + L) * 512]
                                        .rearrange("p (l x) -> p l x", x=512)
                                        [:, :, kt * 128:(kt + 1) * 128],
                                        start=(kt == 0), stop=(kt == 3),
                                    )
                                # cast to fp8 into pt8 (strided dst);
                                # alternate DVE / ScalarE to share the load
                                dst = (pt8_sb[:, t0 * 256:(t0 + L) * 256]
                                       .rearrange("p (l x) -> p l x", x=256)
                                       [:, :, mp * 128:(mp + 1) * 128])
                                src = (pt_ps[:, 0:L * 128]
                                       .rearrange("p (l x) -> p l x", x=128))
                                if (ci * 2 + mp) % 2 == 0:
                                    nc.vector.tensor_copy(dst, src)
                                else:
                                    nc.scalar.copy(dst, src)
                    f8 = None
                    for t in range(T):
                        e = tile_elem[t]
                        pw_ps = pwp.tile([128, QF], F32, tag="pw")
                        lhsT = (pt8_sb[:, t * 256:(t + 1) * 256]
                                .rearrange("p (two m) -> p two m", two=2))
                        wslice = (w8_sb[:, e * 2 * NFEAT:(e + 1) * 2 * NFEAT]
                                  .rearrange("p (two f) -> p two f", two=2))
                        for h in range(2):
                            f0 = q * QF + h * 512
                            nc.tensor.matmul(
                                pw_ps[:, h * 512:(h + 1) * 512],
                                lhsT, wslice[:, :, f0:f0 + 512],
                                start=True, stop=True, perf_mode=DR,
                            )
                        fw = fwp.tile([128, QF], BF16, tag="fw")
                        nc.vector._custom_dve(
                            ADD2_RANGE_WRAP, out=fw[:], in0=pw_ps[:],
                            in1=c_bcq[:, e * QF:(e + 1) * QF],
                            s1=PI, imm2=float(2 * PI),
                        )
                        if t % 2 == 0:
                            f8 = f8p.tile([128, 2 * QF], FP8, tag="f8")
                        nc.scalar.activation(
                            f8[:, (t % 2) * QF:(t % 2 + 1) * QF], fw[:],
                            mybir.ActivationFunctionType.Sin,
                        )
                        if t % 2 == 1:
                            pr = t // 2
                            st_l = (st8_sb[:, pr * 256:(pr + 1) * 256]
                                    .rearrange("p (two m) -> p two m", two=2))
                            f8r = f8[:, :].rearrange("p (two n) -> p two n", two=2)
                            for h in range(2):
                                nc.tensor.matmul(
                                    z_ps[:, h * 512:(h + 1) * 512],
                                    st_l, f8r[:, :, h * 512:(h + 1) * 512],
                                    start=(pr == 0),
                                    stop=(pr == FP - 1 and T % 2 == 0),
                                    perf_mode=DR,
                                )
                        elif t == T - 1:
                            # odd-T tail: single-tile segsum, plain fp8 matmul
                            pr = t // 2
                            for h in range(2):
                                nc.tensor.matmul(
                                    z_ps[:, h * 512:(h + 1) * 512],
                                    st8_sb[:, pr * 256:pr * 256 + 128],
                                    f8[:, h * 512:(h + 1) * 512],
                                    start=(FP == 0), stop=True,
                                )
                    # quarter spill: bf16 on ScalarE (runs right behind the
                    # last sin, no cross-engine bubble), fp8 cast on GPSIMD
                    # (SBUF->SBUF), AG-gating DMA on the ScalarE DMA queue
                    z_bf = zbfp.tile([128, QF], BF16, tag="zbf", name=f"zbf{q}")
                    nc.scalar.copy(z_bf[:], z_ps[:])
                    z8q = z8p.tile([128, QF], FP8, tag="z8")
                    if q == 3:
                        # DVE is free after the last wrap: fastest AG3 gate
                        nc.vector.tensor_copy(z8q[:], z_bf[:])
                    else:
                        nc.gpsimd.tensor_copy(z8q[:], z_bf[:])
                    nc.scalar.dma_start(out=ag_ins[q][:], in_=z8q[:])
                    nc.gpsimd.collective_compute(
                        "AllGather", mybir.AluOpType.bypass,
                        replica_groups=[list(range(NCORES))],
                        ins=[ag_ins[q][:].opt()],
                        outs=[ag_outs[q][:].opt()],
                    )
                    # concat into ag_all as soon as this AG lands; the sync
                    # queue is otherwise idle during phase 1
                    if q < 3:
                        nc.sync.dma_start(out=ag_all[:, q * QF:(q + 1) * QF],
                                          in_=ag_outs[q][:])
                    else:
                        # split the tail concat across both DMA queues
                        nc.sync.dma_start(
                            out=ag_all[0:NMOL // 2, q * QF:(q + 1) * QF],
                            in_=ag_outs[q][0:NMOL // 2, :])
                        nc.scalar.dma_start(
                            out=ag_all[NMOL // 2:, q * QF:(q + 1) * QF],
                            in_=ag_outs[q][NMOL // 2:, :])
                    return z_bf

                with (
                    tc.tile_pool(name="pt", bufs=2, space="PSUM") as ptp,
                    tc.tile_pool(name="zq0", bufs=1, space="PSUM") as zq0p,
                ):
                    zbfs = [quarter(0, zq0p, ptp)]
                with tc.tile_pool(name="zq", bufs=2, space="PSUM") as zqp:
                    zbfs.append(quarter(1, zqp))
                    zbfs.append(quarter(2, zqp))
                    zbfs.append(quarter(3, zqp))
                    # all zty slices now, in the AG3 + AllReduce shadow
                    for q in range(NQ):
                        emit_zty(q, zbfs[q], zqp)
                    nc.scalar.dma_start(out=ary_in[:], in_=zty_bf[:])
                    # zty sum rides the cc stream behind AG3; overlaps with
                    # phase-2 ZTZ compute
                    nc.gpsimd.collective_compute(
                        "AllReduce", mybir.AluOpType.add,
                        replica_groups=[list(range(NCORES))],
                        ins=[ary_in[:].opt()],
                        outs=[ary_out[:].opt()],
                    )

            # ---------------- phase 2: 5 ZTZ blocks + zty ----------------
            with (
                tc.tile_pool(name="zg", bufs=4) as zgp,
                tc.tile_pool(name="osb", bufs=4) as osbp,
                tc.tile_pool(name="pztz", bufs=4, space="PSUM") as pztzp,
            ):
                pids = {nc.sync: nc.sync.partition_id(),
                        nc.scalar: nc.scalar.partition_id()}
                RS = 4 * 1024  # ag_all row stride
                zg = []
                for k, dl in enumerate(DELTAS):
                    zgk = zgp.tile([128, 2 * 2048], FP8, tag="zg", name=f"zg_{k}")
                    for pl in range(2):
                        # src element (m, pp, f) at (2pp+pl)*128*RS + m*RS + f
                        eng = nc.sync if (k * 2 + pl) % 2 == 0 else nc.scalar
                        base = ag_all[0:128, 0:512]
                        dyn = AP(tensor=base.tensor,
                                 offset=((pids[eng] + dl) % NCORES) * 512
                                 + pl * 128 * RS,
                                 ap=[[RS, 128], [256 * RS, 4], [1, 512]])
                        eng.dma_start(
                            out=zgk[:, pl * 2048:(pl + 1) * 2048]
                            .rearrange("p (pp f) -> p pp f", pp=4),
                            in_=dyn)
                    zg.append(zgk[:, :].rearrange("p (two n) -> p two n", two=2))
                for bi, (a, b) in enumerate(POS_BLOCKS):
                    for mm in range(4):
                        ztz_ps = pztzp.tile([128, 512], F32, tag="ztzps")
                        for pp in range(4):
                            nc.tensor.matmul(
                                ztz_ps[:],
                                zg[a][:, :, pp * 512 + mm * 128:
                                      pp * 512 + mm * 128 + 128],
                                zg[b][:, :, pp * 512:(pp + 1) * 512],
                                start=(pp == 0), stop=(pp == 3), perf_mode=DR,
                            )
                        # PSUM -> SBUF on DVE/Act (both idle in phase 2)
                        o_sb = osbp.tile([128, 512], F32, tag="osb")
                        if (bi * 4 + mm) % 2 == 0:
                            nc.vector.tensor_copy(o_sb[:], ztz_ps[:])
                        else:
                            nc.scalar.copy(o_sb[:], ztz_ps[:])
                        nc.sync.dma_start(
                            out=ztz_d[(bi * 4 + mm) * 128:(bi * 4 + mm + 1) * 128, :],
                            in_=o_sb[:],
                        )
                # zty: ship the bf16 AllReduce result; host converts
                nc.scalar.dma_start(out=zty_d[:], in_=ary_out[:])
    nc.finalize()
    return nc


def _prep_inputs(gto, reductors, W_in, b, Y, plan):
    T, P = plan["T"], plan["P"]
    slot_atom = plan["slot_atom"]
    core_mols = plan["core_mols"]

    gto = np.asarray(gto, np.float32)
    red = np.asarray(reductors, np.float32)
    W_np = np.asarray(W_in, np.float32)
    Y_np = np.asarray(Y, np.float32)

    # red_swz[p, ((e*4+kt)*2+mp)*128 + j] = red[e, kt*128+p, mp*128+j]
    red_swz = np.ascontiguousarray(
        red.reshape(NELEM, 4, 128, 2, 128).transpose(2, 0, 1, 3, 4)
    ).reshape(128, NELEM * 4 * 2 * 128).astype(NP_BF16)

    # w8[p, (e*2+pl)*NFEAT + f] = W[e, pl*128+p, f]
    w8 = np.ascontiguousarray(
        W_np.reshape(NELEM, 2, 128, NFEAT).transpose(2, 0, 1, 3)
    ).reshape(128, NELEM * 2 * NFEAT).astype(NP_FP8)

    c_full = np.mod(np.asarray(b, np.float32) + np.pi / 2 + np.pi,
                    2 * np.pi) - np.pi
    # [NELEM, NQ, QF] -> [NQ, NELEM, QF]
    c_swz = np.ascontiguousarray(
        c_full.reshape(NELEM, NQ, QF).transpose(1, 0, 2)
    ).reshape(NQ, 1, NELEM * QF)

    st8_all = plan.get("st8")

    in_maps = []
    for c in range(NCORES):
        sl = slot_atom[c]
        g = np.zeros((T * 128, REP), np.float32)
        real = sl >= 0
        g[real] = gto[sl[real]]
        # [T,128p,512] -> [128p, T*512]
        gto_swz = np.ascontiguousarray(
            g.reshape(T, 128, 4, 128).transpose(3, 0, 2, 1)
        ).reshape(128, T * 512).astype(NP_BF16)
        # st [P,128,256] -> [128, P*256]
        st_swz = np.ascontiguousarray(
            st8_all[c].transpose(1, 0, 2)).reshape(128, P * 256)
        y_swz = Y_np[core_mols[c], 0].reshape(128, 1).astype(NP_BF16)
        in_maps.append({
            "gto_swz": gto_swz,
            "st_swz": st_swz,
            "red_swz": red_swz,
            "w_swz": w8,
            "c_swz": c_swz.astype(NP_BF16),
            "y_swz": y_swz,
        })
    return in_maps


def _get_built(charges, molIDs):
    key = (hash(np.asarray(charges).tobytes()), hash(np.asarray(molIDs).tobytes()))
    if key not in _cache:
        plan = _plan(charges, molIDs)
        nc = _build(plan)
        _cache[key] = (plan, nc)
    return _cache[key]


def run(gto, reductors, W, b, Y, charges, molIDs, trace=False, tmpdir=None):
    plan, nc = _get_built(charges, molIDs)
    in_maps = _prep_inputs(gto, reductors, W, b, Y, plan)
    res = bass_utils.run_bass_kernel_spmd(
        nc, in_maps, core_ids=list(range(NCORES)), trace=trace, tmpdir=tmpdir,
    )
    scale2 = np.float32(2.0 / NFEAT)
    scale = np.float32(np.sqrt(2.0 / NFEAT))
    S = plan["S"]
    ztz = np.zeros((NFEAT, NFEAT), np.float32)
    for d in range(NCORES):
        blocks = res.results[d]["ztz"]
        for bi, (a, bpos) in enumerate(POS_BLOCKS):
            i, j = S[d][a], S[d][bpos]
            blk = blocks[bi * 512:(bi + 1) * 512, :]
            ztz[i * 512:(i + 1) * 512, j * 512:(j + 1) * 512] = blk
            ztz[j * 512:(j + 1) * 512, i * 512:(i + 1) * 512] = blk.T
    ztz *= scale2
    ztz[np.arange(NFEAT), np.arange(NFEAT)] += np.float32(LLAMBDA)
    zty = res.results[0]["zty"][0].astype(np.float32) * scale
    out = np.concatenate([ztz, zty[:, None]], axis=1).astype(np.float32)
    return out, res


def kernel(gto, reductors, W, b, Y, charges, molIDs):
    out, _ = run(gto, reductors, W, b, Y, charges, molIDs)
    return out


# revision 42
# speedup vs baseline: 1.1429x; 1.1429x over previous
"""Trainium2 Bass kernel for the MoE-routing random-feature ridge problem.

Strategy (8 NeuronCores, atom-sharded phase 1 + covering-design phase 2):
  - Molecules are assigned to cores (128 each) by a greedy balance of
    per-element atom counts, so each core gets ~512 atoms of each element
    and elem groups pad to T ~ 19 tiles of 128 atoms.
  - Phase 1 per core, in 4 feature-quarter passes (1024 cols each):
      PT  = reductors[e]^T @ gto^T    bf16 -> fp8     [256, 128] per tile
      PW  = PT^T @ W[e] quarter       fp8 DoubleRow   [128, 1024] psum
      fw  = wrap(PW + c) into [-pi,pi]  custom DVE    bf16
      F   = sin(fw)                   ScalarE         fp8
      Z  += ST^T @ F per tile-pair    fp8 DoubleRow   psum accumulate
    where c = wrap(b + pi/2), so sin(x + c) = cos(x + b).
    Z spills straight to fp8 on GPSIMD (keeps ScalarE for sins); the
    per-quarter zty slice comes from the fp8 Z via tiny PE matmuls with
    fp8 y, and rides as 8 extra bitcast rows in quarter 3's AllGather.
  - Per-quarter AllGathers are triggered from gpsimd as soon as each z8
    spill lands; ag_all concat DMAs ride the ScalarE DMA queue so they
    don't head-of-line-block the spills.
  - Phase 2: each core d reads the feature slices {d, d+1, d+2, d+4}
    (mod 8) of the full Z (fp8) via partition-id dynamic APs and runs the
    same program: 5 [512,512] blocks of Z^T Z with fp8 DoubleRow over 4
    mol-tile pairs; the 8 slice quadruples cover all 36 upper-triangle
    blocks; ZTZ psum DMAs straight to DRAM; host mirrors.
  - Host applies scale^2 = 2/NFEAT, adds lambda*I, assembles + mirrors.
"""

import sys

if "/opt/trn_rl_repo" not in sys.path:
    sys.path.insert(0, "/opt/trn_rl_repo")

import numpy as np

import concourse.bacc as bacc
import concourse.mybir as mybir
import concourse.tile as tile
from concourse import bass_utils
from concourse.ap import AP

NCORES = 8
NATOMS = 16384
NMOL = 1024
REP = 512
PROJ = 256
NFEAT = 4096
NELEM = 4
LLAMBDA = 1e-6
MPC = NMOL // NCORES      # mols per core (128)
NQ = 4                    # feature quarters
QF = NFEAT // NQ          # 1024
DELTAS = (0, 1, 4, 2)
POS_BLOCKS = [(0, 0), (0, 1), (1, 2), (2, 3), (0, 2)]
NBLK = len(POS_BLOCKS)
ZR = 8                    # zty payload rows (bf16 [1,4096] as fp8 [8,1024])
CR = MPC + ZR             # quarter-3 a2a rows

F32 = mybir.dt.float32
BF16 = mybir.dt.bfloat16
FP8 = mybir.dt.float8e4
NP_FP8 = mybir.dt.np(FP8)
NP_BF16 = mybir.dt.np(BF16)

# --- fused (in0 + in1) + range-wrap custom DVE op ---------------------------
from concourse import dve_ops as _dve_ops
from concourse.dve_spec import Spec as _Spec, Src0 as _Src0, Src1 as _Src1
from concourse.dve_spec import C1 as _C1, C2 as _C2, _has_src1, lower as _dve_lower
from concourse.dve_uop import DveOpSpec as _DveOpSpec

_A2RW_NAME = "ADD2_RANGE_WRAP_ANT"
if _A2RW_NAME not in _dve_ops._SUB_OPCODE_FOR_NAME:
    _y = _Src0 + _Src1
    _a2_spec = _Spec(
        body=_y + _C2 * ((_y < -_C1) - (_y > _C1)),
        reference=lambda in0, in1, s0, s1, imm2: (in0 + in1)
        + imm2
        * (
            ((in0 + in1) < -s1).astype(np.float32)
            - ((in0 + in1) > s1).astype(np.float32)
        ),
    )
    _shas = {}
    for _ver in ("v3", "v4"):
        _tmp = _DveOpSpec(name=_A2RW_NAME, opcode=1,
                          uops=_dve_lower(_a2_spec, ver=_ver),
                          rd1_en=_has_src1(_a2_spec))
        _shas[_ver] = _tmp.sha(_ver)
    ADD2_RANGE_WRAP = _dve_ops.DveOp(_A2RW_NAME, _a2_spec, subdim=False, uops_sha=_shas)
    _dve_ops.OPS.append(ADD2_RANGE_WRAP)
    _dve_ops.CUSTOM_DVE_SPECS[_A2RW_NAME] = _a2_spec
    _dve_ops._SUB_OPCODE_FOR_NAME[_A2RW_NAME] = (
        max(_dve_ops._SUB_OPCODE_FOR_NAME.values()) + 1
    )
else:
    ADD2_RANGE_WRAP = next(o for o in _dve_ops.OPS if o.name == _A2RW_NAME)

_cache = {}


def _plan(charges, molIDs):
    charges = np.asarray(charges)
    molIDs = np.asarray(molIDs)
    assert np.all(np.diff(molIDs) >= 0)

    cnt = np.zeros((NMOL, NELEM), np.int64)
    np.add.at(cnt, (molIDs, charges), 1)

    # greedy balanced mol->core assignment (capacity 128 mols per core)
    order = np.argsort(-cnt.sum(1), kind="stable")
    load = np.zeros((NCORES, NELEM), np.float64)
    nmol = np.zeros(NCORES, np.int64)
    core_of = np.zeros(NMOL, np.int64)
    for m in order:
        best, bestJ = -1, None
        for c in range(NCORES):
            if nmol[c] >= MPC:
                continue
            J = float(((load[c] + cnt[m]) ** 2).sum())
            if bestJ is None or J < bestJ:
                best, bestJ = c, J
        core_of[m] = best
        load[best] += cnt[m]
        nmol[best] += 1
    assert np.all(nmol == MPC)

    # hill-climb refinement: swap mols between cores to reduce
    # T = sum_e max_c ceil(cnt[c,e]/128), tiebreak sum_e max_c cnt[c,e]
    icnt = np.zeros((NCORES, NELEM), np.int64)
    for c in range(NCORES):
        icnt[c] = cnt[core_of == c].sum(axis=0)

    def loss(ic):
        mx = ic.max(axis=0)
        return (int(np.ceil(mx / 128.0).sum()) * 1000000 + int(mx.sum()))

    rng = np.random.default_rng(12345)
    cur = loss(icnt)
    mols_by_core = [list(np.nonzero(core_of == c)[0]) for c in range(NCORES)]
    for _ in range(20000):
        c1, c2 = rng.integers(0, NCORES, 2)
        if c1 == c2:
            continue
        m1 = mols_by_core[c1][int(rng.integers(0, MPC))]
        m2 = mols_by_core[c2][int(rng.integers(0, MPC))]
        d1, d2 = cnt[m1], cnt[m2]
        icnt[c1] += d2 - d1
        icnt[c2] += d1 - d2
        new = loss(icnt)
        if new <= cur:
            cur = new
            core_of[m1], core_of[m2] = c2, c1
            mols_by_core[c1].remove(m1); mols_by_core[c1].append(m2)
            mols_by_core[c2].remove(m2); mols_by_core[c2].append(m1)
        else:
            icnt[c1] -= d2 - d1
            icnt[c2] -= d1 - d2
    core_mols = [np.nonzero(core_of == c)[0] for c in range(NCORES)]

    # per-core per-element atom lists and global tile counts
    icnt = np.zeros((NCORES, NELEM), np.int64)
    for c in range(NCORES):
        for e in range(NELEM):
            icnt[c, e] = int(cnt[core_mols[c], e].sum())
    T_e = [int(np.ceil(icnt[:, e].max() / 128)) for e in range(NELEM)]
    T = sum(T_e)
    FP = T // 2          # full DoubleRow pairs
    P = (T + 1) // 2     # st8 pair-slot count
    tile_elem = []
    for e in range(NELEM):
        tile_elem += [e] * T_e[e]

    # proj chunks: runs of same-element tiles, up to 4 tiles each
    chunks = []  # (t0, L, e)
    t = 0
    for e in range(NELEM):
        left = T_e[e]
        while left > 0:
            L = min(4, left)
            chunks.append((t, L, e))
            t += L
            left -= L

    # per-core slot table [T*128] -> atom index or -1; local mol index
    slot_atom = np.full((NCORES, T * 128), -1, np.int64)
    mol_loc = np.full(NMOL, -1, np.int64)
    for c in range(NCORES):
        for i, m in enumerate(core_mols[c]):
            mol_loc[m] = i
        t0 = 0
        core_mask = core_of[molIDs] == c
        for e in range(NELEM):
            idx = np.nonzero(core_mask & (charges == e))[0]
            slot_atom[c, t0 * 128: t0 * 128 + len(idx)] = idx
            t0 += T_e[e]

    # ST (fp8) per pair: [P, 128, 2*128]
    st8 = np.zeros((NCORES, P, 128, 256), dtype=NP_FP8)
    for c in range(NCORES):
        sl = slot_atom[c]
        real = np.nonzero(sl >= 0)[0]
        ml = mol_loc[molIDs[sl[real]]]
        tt = real // 128
        ii = real % 128
        st8[c, tt // 2, ii, (tt % 2) * 128 + ml] = 1.0

    # covering design + host assembly map
    S = [[(d + dl) % NCORES for dl in DELTAS] for d in range(NCORES)]
    cover = set()
    for d in range(NCORES):
        for (a, b) in POS_BLOCKS:
            i, j = S[d][a], S[d][b]
            cover.add((min(i, j), max(i, j)))
    assert len(cover) == 36, f"coverage {len(cover)}"

    return dict(core_mols=core_mols, T_e=T_e, T=T, P=P, FP=FP,
                tile_elem=tile_elem, chunks=chunks, slot_atom=slot_atom,
                S=S, st8=st8)


def _build(plan):
    T, P = plan["T"], plan["P"]
    FP = plan["FP"]
    tile_elem = plan["tile_elem"]
    chunks = plan["chunks"]

    nc = bacc.Bacc(num_devices=NCORES)
    gto_d = nc.dram_tensor("gto_swz", [128, T * 512], BF16, kind="ExternalInput")
    st_d = nc.dram_tensor("st_swz", [128, P * 256], FP8, kind="ExternalInput")
    red_d = nc.dram_tensor("red_swz", [128, NELEM * 4 * 2 * 128], BF16,
                           kind="ExternalInput")
    w_d = nc.dram_tensor("w_swz", [128, NELEM * 2 * NFEAT], FP8,
                         kind="ExternalInput")
    c_d = nc.dram_tensor("c_swz", [NQ, 1, NELEM * QF], BF16, kind="ExternalInput")
    y_d = nc.dram_tensor("y_swz", [128, 1], BF16, kind="ExternalInput")
    ztz_d = nc.dram_tensor("ztz", [NBLK * 4 * 128, 512], F32, kind="ExternalOutput")
    zty_d = nc.dram_tensor("zty", [1, NFEAT], BF16, kind="ExternalOutput")

    PI = float(np.pi)
    e0 = tile_elem[0]

    with tile.TileContext(nc) as tc:
        with (
            tc.tile_pool(name="const", bufs=1) as constp,
            tc.tile_pool(name="zacc", bufs=1) as zaccp,
            tc.tile_pool(name="dram", bufs=1, space="DRAM") as dramp,
        ):
            # priority order: everything the first PW tile needs comes first.
            # c broadcasts are 0-stride replicating DMAs (gpsimd is still
            # busy loading its library this early)
            def bcast_c(dst_ap, q, col0, cols, eng):
                src = AP(tensor=c_d, offset=q * (NELEM * QF) + col0,
                         ap=[[0, 128], [1, cols]])
                eng.dma_start(out=dst_ap, in_=src)

            # sync ring: gto (gates PT) first, then red, then the rest
            gto_sb = constp.tile([128, T * 512], BF16, tag="gto")
            nc.sync.dma_start(out=gto_sb[:, 0:4 * 512], in_=gto_d[:, 0:4 * 512])
            red_sb = constp.tile([128, NELEM * 4 * 2 * 128], BF16, tag="red")
            nc.sync.dma_start(out=red_sb[:], in_=red_d[:])
            nc.sync.dma_start(out=gto_sb[:, 4 * 512:], in_=gto_d[:, 4 * 512:])
            # scalar ring: c broadcast (first wrap) + quarter-0 weights
            c_bcq0 = constp.tile([128, NELEM * QF], BF16, tag="cbq0", name="cbq0")
            c_bcqs = [c_bcq0]
            bcast_c(c_bcq0[:, e0 * QF:(e0 + 1) * QF], 0, e0 * QF, QF, nc.scalar)
            w8_sb = constp.tile([128, NELEM * 2 * NFEAT], FP8, tag="w8")
            for pl in range(2):
                base = (e0 * 2 + pl) * NFEAT
                nc.scalar.dma_start(out=w8_sb[:, base:base + QF],
                                    in_=w_d[:, base:base + QF])
            st8_sb = constp.tile([128, P * 256], FP8, tag="st8")
            nc.scalar.dma_start(out=st8_sb[:, 0:512], in_=st_d[:, 0:512])
            # warmup collective: absorbs first-collective setup + core skew
            warm_in = dramp.tile([1, 64], FP8, tag="warmin")
            warm_out = dramp.tile([NCORES, 64], FP8, addr_space="Shared",
                                  tag="warmout", name="warm_b")
            nc.gpsimd.collective_compute(
                "AllGather", mybir.AluOpType.bypass,
                replica_groups=[list(range(NCORES))],
                ins=[warm_in[:].opt()],
                outs=[warm_out[:].opt()],
            )
            # rest of quarter 0 weights + remaining c broadcasts
            for e in range(NELEM):
                if e == e0:
                    continue
                for pl in range(2):
                    base = (e * 2 + pl) * NFEAT
                    nc.scalar.dma_start(out=w8_sb[:, base:base + QF],
                                        in_=w_d[:, base:base + QF])
            for e in range(NELEM):
                if e == e0:
                    continue
                bcast_c(c_bcq0[:, e * QF:(e + 1) * QF], 0, e * QF, QF,
                        nc.scalar)
            # later-quarter broadcasts: replicating DMAs on the scalar ring
            for q in range(1, NQ):
                c_bcq = constp.tile([128, NELEM * QF], BF16, tag=f"cbq{q}",
                                    name=f"cbq{q}")
                bcast_c(c_bcq[:], q, 0, NELEM * QF, nc.scalar)
                c_bcqs.append(c_bcq)
            y_sb = constp.tile([128, 1], BF16, tag="ybf")
            nc.scalar.dma_start(out=y_sb[:], in_=y_d[:])
            nc.sync.dma_start(out=st8_sb[:, 512:], in_=st_d[:, 512:])
            for e in range(NELEM):
                for pl in range(2):
                    base = (e * 2 + pl) * NFEAT
                    nc.sync.dma_start(out=w8_sb[:, base + QF:base + NFEAT],
                                      in_=w_d[:, base + QF:base + NFEAT])

            pt8_sb = zaccp.tile([128, T * 256], FP8, tag="pt8")
            zty_bf = zaccp.tile([1, NFEAT], BF16, tag="ztybf")

            ag_ins, ag_outs = [], []
            for q in range(NQ):
                ag_ins.append(dramp.tile([MPC, QF], FP8, tag=f"agin{q}",
                                         name=f"agin{q}"))
                ag_outs.append(dramp.tile([NCORES * MPC, QF], FP8,
                                          addr_space="Shared", tag=f"agout{q}",
                                          name=f"ag_b{q}"))
            # mod-8 dynamic addressing in phase 2: no wraparound dups needed
            ag_all = dramp.tile([NMOL, 4 * 1024], FP8, tag="agall")
            ary_in = dramp.tile([1, NFEAT], BF16, tag="aryin")
            ary_out = dramp.tile([1, NFEAT], BF16, addr_space="Shared",
                                 tag="aryout", name="ary_b")

            DR = mybir.MatmulPerfMode.DoubleRow

            with (
                tc.tile_pool(name="pw", bufs=2, space="PSUM") as pwp,
                tc.tile_pool(name="fw", bufs=6) as fwp,
                tc.tile_pool(name="f8", bufs=3) as f8p,
                tc.tile_pool(name="z8", bufs=2) as z8p,
                tc.tile_pool(name="zbf", bufs=4) as zbfp,
            ):
                def emit_zty(q, z_bf, zpool):
                    # borrow a freed z-slot; tiny PE mms from the bf16 Z
                    zty_ps = zpool.tile([128, QF], F32, tag="z",
                                        name=f"ztyps{q}")
                    for h in range(2):
                        nc.tensor.matmul(
                            zty_ps[0:1, h * 512:(h + 1) * 512],
                            y_sb[:], z_bf[:, h * 512:(h + 1) * 512],
                            start=True, stop=True,
                        )
                    nc.scalar.copy(zty_bf[:, q * QF:(q + 1) * QF],
                                   zty_ps[0:1, 0:QF])

                def quarter(q, zpool, ptp=None):
                    c_bcq = c_bcqs[q]
                    z_ps = zpool.tile([128, QF], F32, tag="z", name=f"z_q{q}")
                    if q == 0:
                        for ci, (t0, L, e) in enumerate(chunks):
                            for mp in range(2):
                                pt_ps = ptp.tile([128, 512], F32, tag="pt")
                                for kt in range(4):
                                    nc.tensor.matmul(
                                        pt_ps[:, 0:L * 128],
                                        red_sb[:, ((e * 4 + kt) * 2 + mp) * 128:
                                                  ((e * 4 + kt) * 2 + mp) * 128 + 128],
                                        gto_sb[:, t0 * 512:(t0 + L) * 512]
                                        .rearrange("p (l x) -> p l x", x=512)
                                        [:, :, kt * 128:(kt + 1) * 128],
                                        start=(kt == 0), stop=(kt == 3),
                                    )
                                # cast to fp8 into pt8 (strided dst);
                                # alternate DVE / ScalarE to share the load
                                dst = (pt8_sb[:, t0 * 256:(t0 + L) * 256]
                                       .rearrange("p (l x) -> p l x", x=256)
                                       [:, :, mp * 128:(mp + 1) * 128])
                                src = (pt_ps[:, 0:L * 128]
                                       .rearrange("p (l x) -> p l x", x=128))
                                if (ci * 2 + mp) % 2 == 0:
                                    nc.vector.tensor_copy(dst, src)
                                else:
                                    nc.scalar.copy(dst, src)
                    f8 = None
                    for t in range(T):
                        e = tile_elem[t]
                        pw_ps = pwp.tile([128, QF], F32, tag="pw")
                        lhsT = (pt8_sb[:, t * 256:(t + 1) * 256]
                                .rearrange("p (two m) -> p two m", two=2))
                        wslice = (w8_sb[:, e * 2 * NFEAT:(e + 1) * 2 * NFEAT]
                                  .rearrange("p (two f) -> p two f", two=2))
                        for h in range(2):
                            f0 = q * QF + h * 512
                            nc.tensor.matmul(
                                pw_ps[:, h * 512:(h + 1) * 512],
                                lhsT, wslice[:, :, f0:f0 + 512],
                                start=True, stop=True, perf_mode=DR,
                            )
                        fw = fwp.tile([128, QF], BF16, tag="fw")
                        nc.vector._custom_dve(
                            ADD2_RANGE_WRAP, out=fw[:], in0=pw_ps[:],
                            in1=c_bcq[:, e * QF:(e + 1) * QF],
                            s1=PI, imm2=float(2 * PI),
                        )
                        if t % 2 == 0:
                            f8 = f8p.tile([128, 2 * QF], FP8, tag="f8")
                        nc.scalar.activation(
                            f8[:, (t % 2) * QF:(t % 2 + 1) * QF], fw[:],
                            mybir.ActivationFunctionType.Sin,
                        )
                        if t % 2 == 1:
                            pr = t // 2
                            st_l = (st8_sb[:, pr * 256:(pr + 1) * 256]
                                    .rearrange("p (two m) -> p two m", two=2))
                            f8r = f8[:, :].rearrange("p (two n) -> p two n", two=2)
                            for h in range(2):
                                nc.tensor.matmul(
                                    z_ps[:, h * 512:(h + 1) * 512],
                                    st_l, f8r[:, :, h * 512:(h + 1) * 512],
                                    start=(pr == 0),
                                    stop=(pr == FP - 1 and T % 2 == 0),
                                    perf_mode=DR,
                                )
                        elif t == T - 1:
                            # odd-T tail: single-tile segsum, plain fp8 matmul
                            pr = t // 2
                            for h in range(2):
                                nc.tensor.matmul(
                                    z_ps[:, h * 512:(h + 1) * 512],
                                    st8_sb[:, pr * 256:pr * 256 + 128],
                                    f8[:, h * 512:(h + 1) * 512],
                                    start=(FP == 0), stop=True,
                                )
                    # quarter spill: bf16 on ScalarE (runs right behind the
                    # last sin, no cross-engine bubble), fp8 cast on GPSIMD
                    # (SBUF->SBUF), AG-gating DMA on the ScalarE DMA queue
                    z_bf = zbfp.tile([128, QF], BF16, tag="zbf", name=f"zbf{q}")
                    nc.scalar.copy(z_bf[:], z_ps[:])
                    z8q = z8p.tile([128, QF], FP8, tag="z8")
                    if q == 3:
                        # DVE is free after the last wrap: fastest AG3 gate
                        nc.vector.tensor_copy(z8q[:], z_bf[:])
                    else:
                        nc.gpsimd.tensor_copy(z8q[:], z_bf[:])
                    nc.sync.dma_start(out=ag_ins[q][:], in_=z8q[:])
                    nc.gpsimd.collective_compute(
                        "AllGather", mybir.AluOpType.bypass,
                        replica_groups=[list(range(NCORES))],
                        ins=[ag_ins[q][:].opt()],
                        outs=[ag_outs[q][:].opt()],
                    )
                    # concat into ag_all as soon as this AG lands; the sync
                    # queue is otherwise idle during phase 1
                    if q < 3:
                        nc.sync.dma_start(out=ag_all[:, q * QF:(q + 1) * QF],
                                          in_=ag_outs[q][:])
                    else:
                        # split the tail concat across both DMA queues
                        nc.sync.dma_start(
                            out=ag_all[0:NMOL // 2, q * QF:(q + 1) * QF],
                            in_=ag_outs[q][0:NMOL // 2, :])
                        nc.scalar.dma_start(
                            out=ag_all[NMOL // 2:, q * QF:(q + 1) * QF],
                            in_=ag_outs[q][NMOL // 2:, :])
                    return z_bf

                with (
                    tc.tile_pool(name="pt", bufs=2, space="PSUM") as ptp,
                    tc.tile_pool(name="zq0", bufs=1, space="PSUM") as zq0p,
                ):
                    zbfs = [quarter(0, zq0p, ptp)]
                with tc.tile_pool(name="zq", bufs=2, space="PSUM") as zqp:
                    zbfs.append(quarter(1, zqp))
                    zbfs.append(quarter(2, zqp))
                    zbfs.append(quarter(3, zqp))
                    # all zty slices now, in the AG3 + AllReduce shadow
                    for q in range(NQ):
                        emit_zty(q, zbfs[q], zqp)
                    nc.scalar.dma_start(out=ary_in[:], in_=zty_bf[:])
                    # zty sum rides the cc stream behind AG3; overlaps with
                    # phase-2 ZTZ compute
                    nc.gpsimd.collective_compute(
                        "AllReduce", mybir.AluOpType.add,
                        replica_groups=[list(range(NCORES))],
                        ins=[ary_in[:].opt()],
                        outs=[ary_out[:].opt()],
                    )

            # ---------------- phase 2: 5 ZTZ blocks + zty ----------------
            with (
                tc.tile_pool(name="zg", bufs=4) as zgp,
                tc.tile_pool(name="osb", bufs=4) as osbp,
                tc.tile_pool(name="pztz", bufs=4, space="PSUM") as pztzp,
            ):
                pids = {nc.sync: nc.sync.partition_id(),
                        nc.scalar: nc.scalar.partition_id()}
                RS = 4 * 1024  # ag_all row stride
                zg = []
                for k, dl in enumerate(DELTAS):
                    zgk = zgp.tile([128, 2 * 2048], FP8, tag="zg", name=f"zg_{k}")
                    for pl in range(2):
                        # src element (m, pp, f) at (2pp+pl)*128*RS + m*RS + f
                        eng = nc.sync if (k * 2 + pl) % 2 == 0 else nc.scalar
                        base = ag_all[0:128, 0:512]
                        dyn = AP(tensor=base.tensor,
                                 offset=((pids[eng] + dl) % NCORES) * 512
                                 + pl * 128 * RS,
                                 ap=[[RS, 128], [256 * RS, 4], [1, 512]])
                        eng.dma_start(
                            out=zgk[:, pl * 2048:(pl + 1) * 2048]
                            .rearrange("p (pp f) -> p pp f", pp=4),
                            in_=dyn)
                    zg.append(zgk[:, :].rearrange("p (two n) -> p two n", two=2))
                for bi, (a, b) in enumerate(POS_BLOCKS):
                    for mm in range(4):
                        ztz_ps = pztzp.tile([128, 512], F32, tag="ztzps")
                        for pp in range(4):
                            nc.tensor.matmul(
                                ztz_ps[:],
                                zg[a][:, :, pp * 512 + mm * 128:
                                      pp * 512 + mm * 128 + 128],
                                zg[b][:, :, pp * 512:(pp + 1) * 512],
                                start=(pp == 0), stop=(pp == 3), perf_mode=DR,
                            )
                        # PSUM -> SBUF on DVE/Act (both idle in phase 2)
                        o_sb = osbp.tile([128, 512], F32, tag="osb")
                        if (bi * 4 + mm) % 2 == 0:
                            nc.vector.tensor_copy(o_sb[:], ztz_ps[:])
                        else:
                            nc.scalar.copy(o_sb[:], ztz_ps[:])
                        nc.sync.dma_start(
                            out=ztz_d[(bi * 4 + mm) * 128:(bi * 4 + mm + 1) * 128, :],
                            in_=o_sb[:],
                        )
                # zty: ship the bf16 AllReduce result; host converts
                nc.scalar.dma_start(out=zty_d[:], in_=ary_out[:])
    nc.finalize()
    return nc


def _prep_inputs(gto, reductors, W_in, b, Y, plan):
    T, P = plan["T"], plan["P"]
    slot_atom = plan["slot_atom"]
    core_mols = plan["core_mols"]

    gto = np.asarray(gto, np.float32)
    red = np.asarray(reductors, np.float32)
    W_np = np.asarray(W_in, np.float32)
    Y_np = np.asarray(Y, np.float32)

    # red_swz[p, ((e*4+kt)*2+mp)*128 + j] = red[e, kt*128+p, mp*128+j]
    red_swz = np.ascontiguousarray(
        red.reshape(NELEM, 4, 128, 2, 128).transpose(2, 0, 1, 3, 4)
    ).reshape(128, NELEM * 4 * 2 * 128).astype(NP_BF16)

    # w8[p, (e*2+pl)*NFEAT + f] = W[e, pl*128+p, f]
    w8 = np.ascontiguousarray(
        W_np.reshape(NELEM, 2, 128, NFEAT).transpose(2, 0, 1, 3)
    ).reshape(128, NELEM * 2 * NFEAT).astype(NP_FP8)

    c_full = np.mod(np.asarray(b, np.float32) + np.pi / 2 + np.pi,
                    2 * np.pi) - np.pi
    # [NELEM, NQ, QF] -> [NQ, NELEM, QF]
    c_swz = np.ascontiguousarray(
        c_full.reshape(NELEM, NQ, QF).transpose(1, 0, 2)
    ).reshape(NQ, 1, NELEM * QF)

    st8_all = plan.get("st8")

    in_maps = []
    for c in range(NCORES):
        sl = slot_atom[c]
        g = np.zeros((T * 128, REP), np.float32)
        real = sl >= 0
        g[real] = gto[sl[real]]
        # [T,128p,512] -> [128p, T*512]
        gto_swz = np.ascontiguousarray(
            g.reshape(T, 128, 4, 128).transpose(3, 0, 2, 1)
        ).reshape(128, T * 512).astype(NP_BF16)
        # st [P,128,256] -> [128, P*256]
        st_swz = np.ascontiguousarray(
            st8_all[c].transpose(1, 0, 2)).reshape(128, P * 256)
        y_swz = Y_np[core_mols[c], 0].reshape(128, 1).astype(NP_BF16)
        in_maps.append({
            "gto_swz": gto_swz,
            "st_swz": st_swz,
            "red_swz": red_swz,
            "w_swz": w8,
            "c_swz": c_swz.astype(NP_BF16),
            "y_swz": y_swz,
        })
    return in_maps


def _get_built(charges, molIDs):
    key = (hash(np.asarray(charges).tobytes()), hash(np.asarray(molIDs).tobytes()))
    if key not in _cache:
        plan = _plan(charges, molIDs)
        nc = _build(plan)
        _cache[key] = (plan, nc)
    return _cache[key]


def run(gto, reductors, W, b, Y, charges, molIDs, trace=False, tmpdir=None):
    plan, nc = _get_built(charges, molIDs)
    in_maps = _prep_inputs(gto, reductors, W, b, Y, plan)
    res = bass_utils.run_bass_kernel_spmd(
        nc, in_maps, core_ids=list(range(NCORES)), trace=trace, tmpdir=tmpdir,
    )
    scale2 = np.float32(2.0 / NFEAT)
    scale = np.float32(np.sqrt(2.0 / NFEAT))
    S = plan["S"]
    ztz = np.zeros((NFEAT, NFEAT), np.float32)
    for d in range(NCORES):
        blocks = res.results[d]["ztz"]
        for bi, (a, bpos) in enumerate(POS_BLOCKS):
            i, j = S[d][a], S[d][bpos]
            blk = blocks[bi * 512:(bi + 1) * 512, :]
            ztz[i * 512:(i + 1) * 512, j * 512:(j + 1) * 512] = blk
            ztz[j * 512:(j + 1) * 512, i * 512:(i + 1) * 512] = blk.T
    ztz *= scale2
    ztz[np.arange(NFEAT), np.arange(NFEAT)] += np.float32(LLAMBDA)
    zty = res.results[0]["zty"][0].astype(np.float32) * scale
    out = np.concatenate([ztz, zty[:, None]], axis=1).astype(np.float32)
    return out, res


def kernel(gto, reductors, W, b, Y, charges, molIDs):
    out, _ = run(gto, reductors, W, b, Y, charges, molIDs)
    return out
